# revision 7
# baseline (speedup 1.0000x reference)
"""Trainium2 Bass kernel for a 2-layer GraphSAGE (sum aggregation) GNN.

Strategy (8 NeuronCores, SPMD, two launches):
  - Nodes (dst) sharded 12500/core. Edges partitioned by dst owner.
  - Per core, dst nodes are sorted by in-degree (descending) into "ranks";
    ranks tile into 98 blocks of 128 (12544 slots, 44 zero pads).
  - Launch 1: layer-1 aggregation via round-structured [128,1]-indexed
    indirect DMA gathers of 512B x rows, DVE-accumulated into SBUF agg;
    then per tile h = relu(agg @ Wn1 + x @ Ws1 + b1) on PE, and the
    8-wide projections z = h @ Wn2 and o2 = h @ Ws2 + b2.
  - Using segsum(h[src]) @ Wn2 == segsum((h @ Wn2)[src]), only z (8 wide)
    must be exchanged across cores. The host concatenates the per-core z
    shards (collectives are unavailable on this runtime path).
  - Launch 2: layer-2 aggregation gathers 32B z rows with the same round
    structure, adds o2, applies log_softmax; host inverse-permutes rows.

The host side only reshuffles indices / rows (graph partitioning and the
z-shard concat); all feature compute happens on device.
"""

import sys

import numpy as np

sys.path.insert(0, "/opt/trn_rl_repo")

import concourse.bass as bass
import concourse.mybir as mybir
import concourse.tile as tile
from concourse import bacc
from concourse.bass_utils import run_bass_kernel_spmd
from concourse.masks import make_identity

P = 128
N_NODES = 100000
N_CORES = 8
NPC = N_NODES // N_CORES  # 12500
NT = 98  # rank tiles per core
NR = NT * P  # 12544 rank slots per core
NCLS = 8
ZROW_X = N_NODES  # zeros row appended to x gather table
ZROW_Z = NPC  # core-0 pad rank (z value is exactly 0 by construction)
F32 = mybir.dt.float32
I32 = mybir.dt.int32


def _prep_host(x, edge_src, edge_dst):
    """Partition edges by dst core, degree-sort dst ranks, build round-major
    per-tile gather index arrays. Returns per-core arrays + globals."""
    edge_src = np.asarray(edge_src)
    edge_dst = np.asarray(edge_dst)
    core_of = edge_dst // NPC

    orders = []  # per core: rank -> local dst id
    deg_sorted = []  # per core: degree per rank (desc)
    per_core = []
    for k in range(N_CORES):
        m = core_of == k
        s = edge_src[m]
        dl = edge_dst[m] - k * NPC
        deg = np.bincount(dl, minlength=NPC)
        order = np.argsort(-deg, kind="stable")
        rank_of = np.empty(NPC, dtype=np.int64)
        rank_of[order] = np.arange(NPC)
        orders.append(order)
        deg_sorted.append(deg[order])
        per_core.append((s, rank_of[dl]))

    # global z position of each original node (layout of concatenated z)
    zpos = np.empty(N_NODES, dtype=np.int64)
    for k in range(N_CORES):
        zpos[k * NPC + orders[k]] = k * NR + np.arange(NPC)

    # global per-tile round counts (max over cores; >=1)
    R = np.ones(NT, dtype=np.int64)
    for k in range(N_CORES):
        for t in range(NT):
            lead = t * P
            if lead < NPC:
                R[t] = max(R[t], deg_sorted[k][lead])
    off = np.zeros(NT + 1, dtype=np.int64)
    off[1:] = np.cumsum(R)
    TK = int(off[-1])

    I1s, I2s, xTs = [], [], []
    for k in range(N_CORES):
        s, ranks = per_core[k]
        eo = np.argsort(ranks, kind="stable")
        rs = ranks[eo]
        ss = s[eo]
        starts = np.searchsorted(rs, np.arange(NPC))
        occ = np.arange(len(rs)) - starts[rs]
        maxR = int(R.max())
        A1 = np.full((NR, maxR), ZROW_X, dtype=np.int32)
        A2 = np.full((NR, maxR), ZROW_Z, dtype=np.int32)
        A1[rs, occ] = ss
        A2[rs, occ] = zpos[ss]
        I1 = np.empty((P, TK), dtype=np.int32)
        I2 = np.empty((P, TK), dtype=np.int32)
        for t in range(NT):
            blk = slice(t * P, (t + 1) * P)
            I1[:, off[t] : off[t + 1]] = A1[blk, : R[t]]
            I2[:, off[t] : off[t + 1]] = A2[blk, : R[t]]
        I1s.append(I1)
        I2s.append(I2)
        xT = np.zeros((P, NR), dtype=np.float32)
        xT[:, :NPC] = x[k * NPC + orders[k]].T
        xTs.append(np.ascontiguousarray(xT))

    return orders, R, off, TK, I1s, I2s, xTs


def _build_nc1(R, off, TK):
    """Launch 1: layer-1 aggregate + matmuls; outputs z and o2 per core."""
    nc = bacc.Bacc(
        "TRN2", target_bir_lowering=False, debug=False, num_devices=N_CORES
    )
    xg = nc.dram_tensor("xg", [N_NODES + 1, P], F32, kind="ExternalInput").ap()
    xT = nc.dram_tensor("xT", [P, NR], F32, kind="ExternalInput").ap()
    I1 = nc.dram_tensor("I1", [P, TK], I32, kind="ExternalInput").ap()
    W1n = nc.dram_tensor("W1n", [P, P], F32, kind="ExternalInput").ap()
    W1s = nc.dram_tensor("W1s", [P, P], F32, kind="ExternalInput").ap()
    W2n = nc.dram_tensor("W2n", [P, NCLS], F32, kind="ExternalInput").ap()
    W2s = nc.dram_tensor("W2s", [P, NCLS], F32, kind="ExternalInput").ap()
    b1 = nc.dram_tensor("b1", [1, P], F32, kind="ExternalInput").ap()
    b2 = nc.dram_tensor("b2", [1, NCLS], F32, kind="ExternalInput").ap()
    z_k = nc.dram_tensor("z", [P, NT * NCLS], F32, kind="ExternalOutput").ap()
    o2_k = nc.dram_tensor("o2", [P, NT * NCLS], F32, kind="ExternalOutput").ap()

    with tile.TileContext(nc) as tc:
        with (
            tc.tile_pool(name="persist", bufs=1) as pp,
            tc.tile_pool(name="gather", bufs=8) as gp,
            tc.tile_pool(name="work", bufs=3) as wp,
            tc.tile_pool(name="psum", bufs=1, space="PSUM") as psp,
        ):
            w1n = pp.tile([P, P], F32, tag="w1n")
            w1s = pp.tile([P, P], F32, tag="w1s")
            w2n = pp.tile([P, NCLS], F32, tag="w2n")
            w2s = pp.tile([P, NCLS], F32, tag="w2s")
            b1t = pp.tile([1, P], F32, tag="b1")
            b2t = pp.tile([1, NCLS], F32, tag="b2")
            ones = pp.tile([1, P], F32, tag="ones")
            ident = pp.tile([P, P], F32, tag="ident")
            i1t = pp.tile([P, TK], I32, tag="i1")
            xTt = pp.tile([P, NR], F32, tag="xT")
            agg = pp.tile([P, NR], F32, tag="agg")
            zsb = pp.tile([P, NT * NCLS], F32, tag="z")
            o2sb = pp.tile([P, NT * NCLS], F32, tag="o2")

            nc.sync.dma_start(out=w1n[:], in_=W1n[:])
            nc.sync.dma_start(out=w1s[:], in_=W1s[:])
            nc.sync.dma_start(out=w2n[:], in_=W2n[:])
            nc.sync.dma_start(out=w2s[:], in_=W2s[:])
            nc.sync.dma_start(out=b1t[:], in_=b1[:])
            nc.sync.dma_start(out=b2t[:], in_=b2[:])
            nc.sync.dma_start(out=i1t[:], in_=I1[:])
            nc.sync.dma_start(out=xTt[:], in_=xT[:])
            nc.vector.memset(ones[:], 1.0)
            make_identity(nc, ident[:])

            # layer-1 gather+accumulate (tile-major rounds)
            for t in range(NT):
                csl = slice(t * P, (t + 1) * P)
                for r in range(int(R[t])):
                    col = int(off[t]) + r
                    buf = gp.tile([P, P], F32, tag="g1")
                    nc.gpsimd.indirect_dma_start(
                        out=buf[:],
                        out_offset=None,
                        in_=xg[:],
                        in_offset=bass.IndirectOffsetOnAxis(
                            ap=i1t[:, col : col + 1], axis=0
                        ),
                    )
                    if r == 0:
                        nc.vector.tensor_copy(out=agg[:, csl], in_=buf[:])
                    else:
                        nc.vector.tensor_add(
                            out=agg[:, csl], in0=agg[:, csl], in1=buf[:]
                        )

            # per-tile matmuls: h, z, self-path of layer 2
            for t in range(NT):
                csl = slice(t * P, (t + 1) * P)
                zsl = slice(t * NCLS, (t + 1) * NCLS)
                aggT_ps = psp.tile([P, P], F32, tag="aggT_ps")
                nc.tensor.transpose(
                    out=aggT_ps[:], in_=agg[:, csl], identity=ident[:]
                )
                aggT = wp.tile([P, P], F32, tag="aggT")
                nc.vector.tensor_copy(out=aggT[:], in_=aggT_ps[:])
                h_ps = psp.tile([P, P], F32, tag="h_ps")
                nc.tensor.matmul(
                    out=h_ps[:], lhsT=aggT[:], rhs=w1n[:], start=True, stop=False
                )
                nc.tensor.matmul(
                    out=h_ps[:], lhsT=xTt[:, csl], rhs=w1s[:],
                    start=False, stop=False,
                )
                nc.tensor.matmul(
                    out=h_ps[:], lhsT=ones[:1, :], rhs=b1t[:1, :],
                    start=False, stop=True,
                )
                h = wp.tile([P, P], F32, tag="h")
                nc.scalar.activation(
                    out=h[:], in_=h_ps[:], func=mybir.ActivationFunctionType.Relu
                )
                hT_ps = psp.tile([P, P], F32, tag="hT_ps")
                nc.tensor.transpose(out=hT_ps[:], in_=h[:], identity=ident[:])
                hT = wp.tile([P, P], F32, tag="hT")
                nc.vector.tensor_copy(out=hT[:], in_=hT_ps[:])
                z_ps = psp.tile([P, NCLS], F32, tag="z_ps")
                nc.tensor.matmul(
                    out=z_ps[:], lhsT=hT[:], rhs=w2n[:], start=True, stop=True
                )
                nc.vector.tensor_copy(out=zsb[:, zsl], in_=z_ps[:])
                o2_ps = psp.tile([P, NCLS], F32, tag="o2_ps")
                nc.tensor.matmul(
                    out=o2_ps[:], lhsT=hT[:], rhs=w2s[:], start=True, stop=False
                )
                nc.tensor.matmul(
                    out=o2_ps[:], lhsT=ones[:1, :], rhs=b2t[:1, :],
                    start=False, stop=True,
                )
                nc.vector.tensor_copy(out=o2sb[:, zsl], in_=o2_ps[:])

            nc.sync.dma_start(out=z_k, in_=zsb[:])
            nc.sync.dma_start(out=o2_k, in_=o2sb[:])

    nc.compile()
    return nc


def _build_nc2(R, off, TK):
    """Launch 2: layer-2 gather of z rows, add self-path, log_softmax."""
    nc = bacc.Bacc(
        "TRN2", target_bir_lowering=False, debug=False, num_devices=N_CORES
    )
    zf = nc.dram_tensor(
        "zf", [N_CORES * NR, NCLS], F32, kind="ExternalInput"
    ).ap()
    o2_k = nc.dram_tensor("o2", [P, NT * NCLS], F32, kind="ExternalInput").ap()
    I2 = nc.dram_tensor("I2", [P, TK], I32, kind="ExternalInput").ap()
    out = nc.dram_tensor("out", [P, NT * NCLS], F32, kind="ExternalOutput").ap()

    with tile.TileContext(nc) as tc:
        with (
            tc.tile_pool(name="persist", bufs=1) as pp,
            tc.tile_pool(name="gather", bufs=8) as gp,
        ):
            i2t = pp.tile([P, TK], I32, tag="i2")
            o2sb = pp.tile([P, NT * NCLS], F32, tag="o2")
            a2sb = pp.tile([P, NT * NCLS], F32, tag="a2")
            nc.sync.dma_start(out=i2t[:], in_=I2[:])
            nc.sync.dma_start(out=o2sb[:], in_=o2_k[:])

            for t in range(NT):
                zsl = slice(t * NCLS, (t + 1) * NCLS)
                for r in range(int(R[t])):
                    col = int(off[t]) + r
                    buf2 = gp.tile([P, NCLS], F32, tag="g2")
                    nc.gpsimd.indirect_dma_start(
                        out=buf2[:],
                        out_offset=None,
                        in_=zf[:],
                        in_offset=bass.IndirectOffsetOnAxis(
                            ap=i2t[:, col : col + 1], axis=0
                        ),
                    )
                    if r == 0:
                        nc.vector.tensor_copy(out=a2sb[:, zsl], in_=buf2[:])
                    else:
                        nc.vector.tensor_add(
                            out=a2sb[:, zsl], in0=a2sb[:, zsl], in1=buf2[:]
                        )

            nc.vector.tensor_add(out=a2sb[:], in0=a2sb[:], in1=o2sb[:])
            a3 = a2sb[:].rearrange("p (t c) -> p t c", c=NCLS)
            mx = pp.tile([P, NT], F32, tag="mx")
            nc.vector.tensor_reduce(
                out=mx[:], in_=a3, axis=mybir.AxisListType.X,
                op=mybir.AluOpType.max,
            )
            mxb = mx[:].unsqueeze(2).to_broadcast([P, NT, NCLS])
            nc.vector.tensor_tensor(
                out=a3, in0=a3, in1=mxb, op=mybir.AluOpType.subtract
            )
            ex = pp.tile([P, NT * NCLS], F32, tag="ex")
            nc.scalar.activation(
                out=ex[:], in_=a2sb[:], func=mybir.ActivationFunctionType.Exp
            )
            sm = pp.tile([P, NT], F32, tag="sm")
            nc.vector.tensor_reduce(
                out=sm[:],
                in_=ex[:].rearrange("p (t c) -> p t c", c=NCLS),
                axis=mybir.AxisListType.X,
                op=mybir.AluOpType.add,
            )
            lg = pp.tile([P, NT], F32, tag="lg")
            nc.scalar.activation(
                out=lg[:], in_=sm[:], func=mybir.ActivationFunctionType.Ln
            )
            lgb = lg[:].unsqueeze(2).to_broadcast([P, NT, NCLS])
            nc.vector.tensor_tensor(
                out=a3, in0=a3, in1=lgb, op=mybir.AluOpType.subtract
            )
            nc.sync.dma_start(out=out[:], in_=a2sb[:])

    nc.compile()
    return nc


def kernel(
    x, edge_src, edge_dst, W_neigh1, W_self1, b1, W_neigh2, W_self2, b2
):
    x = np.ascontiguousarray(np.asarray(x, dtype=np.float32))
    orders, R, off, TK, I1s, I2s, xTs = _prep_host(x, edge_src, edge_dst)

    xg = np.vstack([x, np.zeros((1, P), np.float32)])
    common = {
        "xg": xg,
        "W1n": np.asarray(W_neigh1, np.float32),
        "W1s": np.asarray(W_self1, np.float32),
        "W2n": np.asarray(W_neigh2, np.float32),
        "W2s": np.asarray(W_self2, np.float32),
        "b1": np.asarray(b1, np.float32).reshape(1, P),
        "b2": np.asarray(b2, np.float32).reshape(1, NCLS),
    }
    in_maps1 = [
        {**common, "xT": xTs[k], "I1": I1s[k]} for k in range(N_CORES)
    ]

    nc1 = _build_nc1(R, off, TK)
    res1 = run_bass_kernel_spmd(nc1, in_maps1, list(range(N_CORES)))

    def _rows(a):  # [P, NT*NCLS] sbuf layout -> [NR, NCLS] rank rows
        return np.ascontiguousarray(
            a.reshape(P, NT, NCLS).transpose(1, 0, 2).reshape(NR, NCLS)
        )

    z_full = np.concatenate(
        [_rows(res1.results[k]["z"]) for k in range(N_CORES)], axis=0
    )
    in_maps2 = [
        {"zf": z_full, "o2": res1.results[k]["o2"], "I2": I2s[k]}
        for k in range(N_CORES)
    ]
    nc2 = _build_nc2(R, off, TK)
    res2 = run_bass_kernel_spmd(nc2, in_maps2, list(range(N_CORES)))

    out_full = np.empty((N_NODES, NCLS), dtype=np.float32)
    for k in range(N_CORES):
        out_full[k * NPC + orders[k]] = _rows(res2.results[k]["out"])[:NPC]
    return out_full


if __name__ == "__main__":
    import jax

    import reference

    cpu = jax.devices("cpu")[0]
    with jax.default_device(cpu):
        inputs = {k: np.asarray(v) for k, v in reference.setup_inputs().items()}
        exp = np.asarray(
            reference.reference(**{k: jax.device_put(v, cpu) for k, v in inputs.items()})
        )
    got = kernel(**inputs)
    err = np.abs(got - exp)
    rel = err / (np.abs(exp) + 1e-6)
    print("max abs err:", err.max(), "max rel err:", rel.max())
